# revision 21
# baseline (speedup 1.0000x reference)
"""Trainium2 Bass kernel for GaussianKernelLayer.

y[n] = sum_m softmax(coef)[m] * norm * exp(-0.5*|x_n - c_m|^2),
N=500000, M=256, D=4, sigma=1. Data-parallel over 8 cores (x sharded on N).

The exp work on the Scalar (ACT) engine is the hard floor: N*M/core =
16.25M elements at 1 elem/cycle @ 1.2 GHz ~= 104 us. Structure:

  - [point, center] layout: psum[pt, 256*a + ctr] holds the full exp
    argument z = x.c + ln(w*norm) - 0.5|c|^2 - 0.5|x|^2 for 8 point-blocks
    (a = 0..7). Stationary = x-features [K=128, 128 pts], 8 blocks stacked
    along K (16 rows each); moving = constant block-diagonal center
    matrix. 4 matmuls of 512 cols per group (ISA cap), PE 32-row tiles.
  - ACT Exp processes centers [0, 256-CC) of each block: [128, 8*(256-CC)]
    PSUM f32 -> fp16 SBUF per group.
  - The last CC centers are offloaded to DVE via a scaled int16
    Schraudolph: u = clamp(A16*z', 0) with A16 = 2^10/log(2) and the bias
    ln(w*norm) pre-shifted by (15 + 12 - 0.0437)*log(2) on the host;
    round(u) as int16 IS the fp16 bit pattern of e^z * 2^12 (the 2^12
    keeps z in [-18.7, -2.8] representable; descale by 2^-12 happens in
    the first reduction add). ~3% per-term sawtooth on 11% of the mass:
    measured rel L2 ~2e-3 (budget 2e-2).
  - Reduction over centers: pairwise fp16 adds (DVE 2x / GpSimd) then one
    tensor_reduce, fused over PAIRS of groups to amortize per-instruction
    overhead; GpSimd carries part of the tree since ACT is the bottleneck.
  - All bias terms folded on host; y written partition-major straight
    from the f32 accumulator tile, drained in overlapped chunks.
"""

import math

import numpy as np

import concourse.bass as bass
import concourse.bacc as bacc_mod
import concourse.mybir as mybir
from concourse.bass_utils import run_bass_kernel_spmd
from concourse.tile import TileContext

N_CORES = 8
N_TOTAL = 500000
PER_CORE = N_TOTAL // N_CORES  # 62500
M = 256
D = 4
SIGMA = 1.0

NG = 62            # groups per core (31 pairs)
BLK = 8            # point-blocks per group
GPTS = 128 * BLK   # 1024 points per group
NP = NG * GPTS     # 63488 padded points per core
SLOTS = NP // 128  # 496 slots per lane
XCOLS = NG * 128   # 7936 stationary columns

CC = 32            # centers per block offloaded to DVE-Schraudolph
MA = M - CC        # centers per block through ACT
SHIFT = 12         # fp16-bits pre-scale (2^SHIFT), descale in first add
A16 = 1024.0 / math.log(2.0)
B16 = 1024.0 * (15.0 + SHIFT - 0.043677448)
DELTA = B16 / A16  # host-side bias pre-shift for offloaded centers

F16 = mybir.dt.float16
I16 = mybir.dt.int16
F32 = mybir.dt.float32

_CACHE = {}


def _build_nc():
    nc = bacc_mod.Bacc()

    # header = block-diag centers [128, 512] + first 128 stationary cols,
    # one DMA so the first matmuls wait on a single small transfer
    hdr_d = nc.dram_tensor("hdr", [128, 2 * M + 128], F16, kind="ExternalInput")
    xs_d = nc.dram_tensor("xs", [128, XCOLS - 128], F16, kind="ExternalInput")
    y_d = nc.dram_tensor("y", [NP], F32, kind="ExternalOutput")

    with TileContext(nc) as tc:
        with (
            tc.tile_pool(name="const", bufs=1) as constp,
            tc.tile_pool(name="xsp", bufs=1) as xsp,
            tc.tile_pool(name="expp", bufs=3) as expp,
            tc.tile_pool(name="redp", bufs=3) as redp,
            tc.tile_pool(name="yp", bufs=1) as yp,
            tc.tile_pool(name="psp", bufs=2, space="PSUM") as psp,
        ):
            hdr = constp.tile([128, 2 * M + 128], F16)
            nc.sync.dma_start(hdr[:], hdr_d[:])
            cd_sb = hdr[:, 0 : 2 * M]

            # remaining stationary stream: small slabs first, then the bulk
            xs1 = xsp.tile([128, 896], F16, tag="xs1")
            nc.sync.dma_start(xs1[:], xs_d[:, 0:896])
            xs2 = xsp.tile([128, XCOLS - 1024], F16, tag="xs2")
            nc.sync.dma_start(xs2[:], xs_d[:, 896:])

            def stat(g):
                c = 128 * g
                if c < 128:
                    return hdr[:, 2 * M : 2 * M + 128]
                if c < 1024:
                    return xs1[:, c - 128 : c]
                return xs2[:, c - 1024 : c - 1024 + 128]

            ys = yp.tile([128, SLOTS], F32, tag="ys")

            drains = {15: (0, 124), 31: (124, 248), 47: (248, 372),
                      59: (372, 480), 61: (480, 496)}

            def tree(g, ex):
                # reduction over centers: fp16 pairwise adds (both exp
                # branches carry the same 2^SHIFT scale so this is uniform).
                # Outputs stay flat/collapsible; t3 rides GpSimd (off the
                # DVE critical queue).
                e3 = ex[:].rearrange("p (a c) -> p a c", c=M)
                t1 = redp.tile([128, BLK * 128], F16, tag="t1")
                nc.vector.tensor_tensor(
                    t1[:], e3[:, :, 0:128], e3[:, :, 128:256],
                    mybir.AluOpType.add,
                )
                t2 = redp.tile([128, BLK * 64], F16, tag="t2")
                v1 = t1[:].rearrange("p (s two c) -> p s two c", two=2, c=64)
                nc.vector.tensor_tensor(
                    t2[:], v1[:, :, 0, :], v1[:, :, 1, :],
                    mybir.AluOpType.add,
                )
                t3 = redp.tile([128, BLK * 32], F16, tag="t3")
                v2 = t2[:].rearrange("p (s two c) -> p s two c", two=2, c=32)
                nc.gpsimd.tensor_tensor(
                    t3[:], v2[:, :, 0, :], v2[:, :, 1, :],
                    mybir.AluOpType.add,
                )
                nc.vector.tensor_reduce(
                    ys[:, BLK * g : BLK * (g + 1)],
                    t3[:].rearrange("p (s c) -> p s c", c=32),
                    axis=mybir.AxisListType.X,
                    op=mybir.AluOpType.add,
                )
                if g in drains:
                    c0, c1 = drains[g]
                    nc.vector.tensor_scalar_mul(
                        ys[:, c0:c1], ys[:, c0:c1], float(2.0 ** -SHIFT)
                    )
                    nc.sync.dma_start(
                        y_d.rearrange("(p f) -> p f", p=128)[:, c0:c1],
                        ys[:, c0:c1],
                    )

            prev = None
            for g in range(NG):
                st = stat(g)
                ps = psp.tile([128, BLK * M], F32, tag="ps")
                for a in range(4):
                    nc.tensor.matmul(
                        ps[:, 512 * a : 512 * (a + 1)],
                        st[32 * a : 32 * a + 32, :],
                        cd_sb[32 * a : 32 * a + 32, :],
                        start=True,
                        stop=True,
                        tile_position=(32 * a, 0),
                    )
                ex = expp.tile([128, BLK * M], F16, tag="ex")
                e3 = ex[:].rearrange("p (a c) -> p a c", c=M)
                e3i = ex[:].bitcast(I16).rearrange("p (a c) -> p a c", c=M)
                p4 = ps[:].rearrange("p (a c) -> p a c", c=M)
                # Schraudolph first: it is the second psum reader, issuing
                # it before the long Exp lets it clear the psum dependency
                # early so the next group's matmuls are never gated on DVE.
                # round(clamp(A16*z', 0)) as int16 IS the fp16 bit pattern
                # of e^z * 2^SHIFT (int16-out converting write).
                nc.vector.tensor_scalar(
                    e3i[:, :, MA:M],
                    p4[:, :, MA:M],
                    float(A16),
                    0.0,
                    mybir.AluOpType.mult,
                    mybir.AluOpType.max,
                )
                nc.scalar.activation(
                    e3[:, :, 0:MA],
                    p4[:, :, 0:MA],
                    mybir.ActivationFunctionType.Exp,
                )
                if prev is not None:
                    tree(*prev)
                prev = (g, ex)
            tree(*prev)
    nc.compile()
    return nc


def _host_prep(x, centers, coefficients):
    """Host-side prep: softmax over 256 coefficients, fp16 hi/lo splits,
    per-center and per-point bias folding, streaming layout."""
    x = np.ascontiguousarray(np.asarray(x, dtype=np.float32))
    centers = np.asarray(centers, dtype=np.float32)
    coefficients = np.asarray(coefficients, dtype=np.float32)

    norm_const = np.float32(1.0 / ((2.0 * math.pi) ** (D / 2) * SIGMA**D))
    e = np.exp(coefficients - coefficients.max())
    w = (e / e.sum()).astype(np.float32)
    b = np.log(w * norm_const).astype(np.float32) - 0.5 * (centers**2).sum(axis=1)
    # uniform 2^SHIFT pre-scale on both branches (descaled before the y
    # drain); offloaded centers additionally carry the Schraudolph shift
    b = b.copy()
    b[:MA] += np.float32(SHIFT * math.log(2.0))
    b[MA:] += np.float32(DELTA)

    cT = centers.T  # [4, 256]
    c_hi = cT.astype(np.float16)
    c_lo = (cT - c_hi.astype(np.float32)).astype(np.float16)
    b_hi = b.astype(np.float16)
    b_lo = (b - b_hi.astype(np.float32)).astype(np.float16)

    crows = np.empty((16, M), dtype=np.float16)
    crows[0:4] = c_hi
    crows[4:8] = c_lo
    crows[8:12] = c_hi
    crows[12:14] = 1.0
    crows[14] = b_hi
    crows[15] = b_lo

    # [32, 512] two-block diagonal, replicated on all four 32-row bands so
    # band q's slice pairs with stationary rows 32q:32q+32 (blocks 2q, 2q+1)
    cd = np.zeros((128, 2 * M), dtype=np.float16)
    for q in range(4):
        cd[32 * q : 32 * q + 16, 0:M] = crows
        cd[32 * q + 16 : 32 * q + 32, M : 2 * M] = crows

    in_maps = []
    for i in range(N_CORES):
        xs = x[i * PER_CORE : (i + 1) * PER_CORE]
        xp = np.zeros((NP, D), dtype=np.float32)
        xp[:PER_CORE] = xs
        xh = xp.astype(np.float16)
        xl = (xp - xh.astype(np.float32)).astype(np.float16)
        sq = -0.5 * (xp * xp).sum(axis=1)
        sq_hi = sq.astype(np.float16)
        sq_lo = (sq - sq_hi.astype(np.float32)).astype(np.float16)

        feat = np.empty((16, NP), dtype=np.float16)
        feat[0:4] = xh.T      # pairs with c_hi
        feat[4:8] = xh.T      # pairs with c_lo
        feat[8:12] = xl.T     # pairs with c_hi
        feat[12] = sq_hi      # pairs with 1
        feat[13] = sq_lo      # pairs with 1
        feat[14:16] = 1.0     # pairs with b_hi / b_lo

        # n = m*496 + 8g + a  ->  stationary[16a + k, g*128 + m] = feat[k, n]
        xsd = (
            feat.reshape(16, 128, NG, BLK)
            .transpose(3, 0, 2, 1)
            .reshape(128, XCOLS)
        )
        hdr = np.concatenate([cd, xsd[:, 0:128]], axis=1)
        in_maps.append(
            {
                "hdr": np.ascontiguousarray(hdr),
                "xs": np.ascontiguousarray(xsd[:, 128:]),
            }
        )
    return in_maps


last_result = None


def kernel(x, centers, coefficients):
    global last_result
    if "nc" not in _CACHE:
        _CACHE["nc"] = _build_nc()
    nc = _CACHE["nc"]
    in_maps = _host_prep(x, centers, coefficients)
    res = run_bass_kernel_spmd(nc, in_maps, core_ids=list(range(N_CORES)))
    last_result = res
    out = []
    for r in res.results:
        y = r["y"][:PER_CORE]
        out.append(y)
    return np.concatenate(out).astype(np.float32)


# revision 22
# speedup vs baseline: 1.0028x; 1.0028x over previous
"""Trainium2 Bass kernel for GaussianKernelLayer.

y[n] = sum_m softmax(coef)[m] * norm * exp(-0.5*|x_n - c_m|^2),
N=500000, M=256, D=4, sigma=1. Data-parallel over 8 cores (x sharded on N).

The exp work on the Scalar (ACT) engine is the hard floor: N*M/core =
16.25M elements at 1 elem/cycle @ 1.2 GHz ~= 104 us. Structure:

  - [point, center] layout: psum[pt, 256*a + ctr] holds the full exp
    argument z = x.c + ln(w*norm) - 0.5|c|^2 - 0.5|x|^2 for 8 point-blocks
    (a = 0..7). Stationary = x-features [K=128, 128 pts], 8 blocks stacked
    along K (16 rows each); moving = constant block-diagonal center
    matrix. 4 matmuls of 512 cols per group (ISA cap), PE 32-row tiles.
  - ACT Exp processes centers [0, 256-CC) of each block: [128, 8*(256-CC)]
    PSUM f32 -> fp16 SBUF per group.
  - The last CC centers are offloaded to DVE via a scaled int16
    Schraudolph: u = clamp(A16*z', 0) with A16 = 2^10/log(2) and the bias
    ln(w*norm) pre-shifted by (15 + 12 - 0.0437)*log(2) on the host;
    round(u) as int16 IS the fp16 bit pattern of e^z * 2^12 (the 2^12
    keeps z in [-18.7, -2.8] representable; descale by 2^-12 happens in
    the first reduction add). ~3% per-term sawtooth on 11% of the mass:
    measured rel L2 ~2e-3 (budget 2e-2).
  - Reduction over centers: pairwise fp16 adds (DVE 2x / GpSimd) then one
    tensor_reduce, fused over PAIRS of groups to amortize per-instruction
    overhead; GpSimd carries part of the tree since ACT is the bottleneck.
  - All bias terms folded on host; y written partition-major straight
    from the f32 accumulator tile, drained in overlapped chunks.
"""

import math

import numpy as np

import concourse.bass as bass
import concourse.bacc as bacc_mod
import concourse.mybir as mybir
from concourse.bass_utils import run_bass_kernel_spmd
from concourse.tile import TileContext

N_CORES = 8
N_TOTAL = 500000
PER_CORE = N_TOTAL // N_CORES  # 62500
M = 256
D = 4
SIGMA = 1.0

NG = 62            # groups per core (31 pairs)
BLK = 8            # point-blocks per group
GPTS = 128 * BLK   # 1024 points per group
NP = NG * GPTS     # 63488 padded points per core
SLOTS = NP // 128  # 496 slots per lane
XCOLS = NG * 128   # 7936 stationary columns

CC = 0             # centers per block offloaded to DVE-Schraudolph
MA = M - CC        # centers per block through ACT
SHIFT = 12         # fp16-bits pre-scale (2^SHIFT), descale in first add
A16 = 1024.0 / math.log(2.0)
B16 = 1024.0 * (15.0 + SHIFT - 0.043677448)
DELTA = B16 / A16  # host-side bias pre-shift for offloaded centers

F16 = mybir.dt.float16
I16 = mybir.dt.int16
F32 = mybir.dt.float32

_CACHE = {}


def _build_nc():
    nc = bacc_mod.Bacc()

    # header = block-diag centers [128, 512] + first 128 stationary cols,
    # one DMA so the first matmuls wait on a single small transfer
    hdr_d = nc.dram_tensor("hdr", [128, 2 * M + 128], F16, kind="ExternalInput")
    xs_d = nc.dram_tensor("xs", [128, XCOLS - 128], F16, kind="ExternalInput")
    y_d = nc.dram_tensor("y", [NP], F32, kind="ExternalOutput")

    with TileContext(nc) as tc:
        with (
            tc.tile_pool(name="const", bufs=1) as constp,
            tc.tile_pool(name="xsp", bufs=1) as xsp,
            tc.tile_pool(name="expp", bufs=3) as expp,
            tc.tile_pool(name="redp", bufs=3) as redp,
            tc.tile_pool(name="yp", bufs=1) as yp,
            tc.tile_pool(name="psp", bufs=2, space="PSUM") as psp,
        ):
            hdr = constp.tile([128, 2 * M + 128], F16)
            nc.sync.dma_start(hdr[:], hdr_d[:])
            cd_sb = hdr[:, 0 : 2 * M]

            # remaining stationary stream: small slabs first, then the bulk
            xs1 = xsp.tile([128, 896], F16, tag="xs1")
            nc.sync.dma_start(xs1[:], xs_d[:, 0:896])
            xs2 = xsp.tile([128, XCOLS - 1024], F16, tag="xs2")
            nc.sync.dma_start(xs2[:], xs_d[:, 896:])

            def stat(g):
                c = 128 * g
                if c < 128:
                    return hdr[:, 2 * M : 2 * M + 128]
                if c < 1024:
                    return xs1[:, c - 128 : c]
                return xs2[:, c - 1024 : c - 1024 + 128]

            ys = yp.tile([128, SLOTS], F32, tag="ys")

            drains = {15: (0, 124), 31: (124, 248), 47: (248, 372),
                      59: (372, 480), 61: (480, 496)}

            def tree(g, ex):
                # reduction over centers: fp16 pairwise adds (both exp
                # branches carry the same 2^SHIFT scale so this is uniform).
                # Outputs stay flat/collapsible; t3 rides GpSimd (off the
                # DVE critical queue).
                e3 = ex[:].rearrange("p (a c) -> p a c", c=M)
                t1 = redp.tile([128, BLK * 128], F16, tag="t1")
                nc.vector.tensor_tensor(
                    t1[:], e3[:, :, 0:128], e3[:, :, 128:256],
                    mybir.AluOpType.add,
                )
                t2 = redp.tile([128, BLK * 64], F16, tag="t2")
                v1 = t1[:].rearrange("p (s two c) -> p s two c", two=2, c=64)
                nc.vector.tensor_tensor(
                    t2[:], v1[:, :, 0, :], v1[:, :, 1, :],
                    mybir.AluOpType.add,
                )
                t3 = redp.tile([128, BLK * 32], F16, tag="t3")
                v2 = t2[:].rearrange("p (s two c) -> p s two c", two=2, c=32)
                nc.gpsimd.tensor_tensor(
                    t3[:], v2[:, :, 0, :], v2[:, :, 1, :],
                    mybir.AluOpType.add,
                )
                nc.vector.tensor_reduce(
                    ys[:, BLK * g : BLK * (g + 1)],
                    t3[:].rearrange("p (s c) -> p s c", c=32),
                    axis=mybir.AxisListType.X,
                    op=mybir.AluOpType.add,
                )
                if g in drains:
                    c0, c1 = drains[g]
                    nc.vector.tensor_scalar_mul(
                        ys[:, c0:c1], ys[:, c0:c1], float(2.0 ** -SHIFT)
                    )
                    nc.sync.dma_start(
                        y_d.rearrange("(p f) -> p f", p=128)[:, c0:c1],
                        ys[:, c0:c1],
                    )

            prev = None
            for g in range(NG):
                st = stat(g)
                ps = psp.tile([128, BLK * M], F32, tag="ps")
                for a in range(4):
                    nc.tensor.matmul(
                        ps[:, 512 * a : 512 * (a + 1)],
                        st[32 * a : 32 * a + 32, :],
                        cd_sb[32 * a : 32 * a + 32, :],
                        start=True,
                        stop=True,
                        tile_position=(32 * a, 0),
                    )
                ex = expp.tile([128, BLK * M], F16, tag="ex")
                e3 = ex[:].rearrange("p (a c) -> p a c", c=M)
                e3i = ex[:].bitcast(I16).rearrange("p (a c) -> p a c", c=M)
                p4 = ps[:].rearrange("p (a c) -> p a c", c=M)
                if CC:
                    # Schraudolph: round(clamp(A16*z', 0)) as int16 IS the
                    # fp16 bit pattern of e^z * 2^SHIFT
                    nc.vector.tensor_scalar(
                        e3i[:, :, MA:M],
                        p4[:, :, MA:M],
                        float(A16),
                        0.0,
                        mybir.AluOpType.mult,
                        mybir.AluOpType.max,
                    )
                nc.scalar.activation(
                    e3[:, :, 0:MA],
                    p4[:, :, 0:MA],
                    mybir.ActivationFunctionType.Exp,
                )
                if prev is not None:
                    tree(*prev)
                prev = (g, ex)
            tree(*prev)
    nc.compile()
    return nc


def _host_prep(x, centers, coefficients):
    """Host-side prep: softmax over 256 coefficients, fp16 hi/lo splits,
    per-center and per-point bias folding, streaming layout."""
    x = np.ascontiguousarray(np.asarray(x, dtype=np.float32))
    centers = np.asarray(centers, dtype=np.float32)
    coefficients = np.asarray(coefficients, dtype=np.float32)

    norm_const = np.float32(1.0 / ((2.0 * math.pi) ** (D / 2) * SIGMA**D))
    e = np.exp(coefficients - coefficients.max())
    w = (e / e.sum()).astype(np.float32)
    b = np.log(w * norm_const).astype(np.float32) - 0.5 * (centers**2).sum(axis=1)
    # uniform 2^SHIFT pre-scale on both branches (descaled before the y
    # drain); offloaded centers additionally carry the Schraudolph shift
    b = b.copy()
    b[:MA] += np.float32(SHIFT * math.log(2.0))
    b[MA:] += np.float32(DELTA)

    cT = centers.T  # [4, 256]
    c_hi = cT.astype(np.float16)
    c_lo = (cT - c_hi.astype(np.float32)).astype(np.float16)
    b_hi = b.astype(np.float16)
    b_lo = (b - b_hi.astype(np.float32)).astype(np.float16)

    crows = np.empty((16, M), dtype=np.float16)
    crows[0:4] = c_hi
    crows[4:8] = c_lo
    crows[8:12] = c_hi
    crows[12:14] = 1.0
    crows[14] = b_hi
    crows[15] = b_lo

    # [32, 512] two-block diagonal, replicated on all four 32-row bands so
    # band q's slice pairs with stationary rows 32q:32q+32 (blocks 2q, 2q+1)
    cd = np.zeros((128, 2 * M), dtype=np.float16)
    for q in range(4):
        cd[32 * q : 32 * q + 16, 0:M] = crows
        cd[32 * q + 16 : 32 * q + 32, M : 2 * M] = crows

    in_maps = []
    for i in range(N_CORES):
        xs = x[i * PER_CORE : (i + 1) * PER_CORE]
        xp = np.zeros((NP, D), dtype=np.float32)
        xp[:PER_CORE] = xs
        xh = xp.astype(np.float16)
        xl = (xp - xh.astype(np.float32)).astype(np.float16)
        sq = -0.5 * (xp * xp).sum(axis=1)
        sq_hi = sq.astype(np.float16)
        sq_lo = (sq - sq_hi.astype(np.float32)).astype(np.float16)

        feat = np.empty((16, NP), dtype=np.float16)
        feat[0:4] = xh.T      # pairs with c_hi
        feat[4:8] = xh.T      # pairs with c_lo
        feat[8:12] = xl.T     # pairs with c_hi
        feat[12] = sq_hi      # pairs with 1
        feat[13] = sq_lo      # pairs with 1
        feat[14:16] = 1.0     # pairs with b_hi / b_lo

        # n = m*496 + 8g + a  ->  stationary[16a + k, g*128 + m] = feat[k, n]
        xsd = (
            feat.reshape(16, 128, NG, BLK)
            .transpose(3, 0, 2, 1)
            .reshape(128, XCOLS)
        )
        hdr = np.concatenate([cd, xsd[:, 0:128]], axis=1)
        in_maps.append(
            {
                "hdr": np.ascontiguousarray(hdr),
                "xs": np.ascontiguousarray(xsd[:, 128:]),
            }
        )
    return in_maps


last_result = None


def kernel(x, centers, coefficients):
    global last_result
    if "nc" not in _CACHE:
        _CACHE["nc"] = _build_nc()
    nc = _CACHE["nc"]
    in_maps = _host_prep(x, centers, coefficients)
    res = run_bass_kernel_spmd(nc, in_maps, core_ids=list(range(N_CORES)))
    last_result = res
    out = []
    for r in res.results:
        y = r["y"][:PER_CORE]
        out.append(y)
    return np.concatenate(out).astype(np.float32)


# revision 23
# speedup vs baseline: 1.2363x; 1.2328x over previous
"""Trainium2 Bass kernel for GaussianKernelLayer.

y[n] = sum_m softmax(coef)[m] * norm * exp(-0.5*|x_n - c_m|^2),
N=500000, M=256, D=4, sigma=1. Data-parallel over 8 cores (x sharded on N).

The exp work on the Scalar (ACT) engine is the hard floor: N*M/core =
16.25M elements at 1 elem/cycle/partition @ 1.2 GHz ~= 104 us. Everything
else is shaped to hide under it:

  - [point, center] layout: psum[pt, 256*a + ctr] holds the full exp
    argument z = x.c + ln(w*norm) - 0.5|c|^2 - 0.5|x|^2 for 8 point-blocks
    (a = 0..7) at once. Stationary = x-features [K=128, 128 pts] with the
    8 blocks STACKED along K (16 rows each: x_hi(4) x_hi(4) x_lo(4)
    sq_hi sq_lo 1 1); moving = a constant block-diagonal center matrix.
    The ISA caps matmul moving free at 512, so 4 matmuls per group using
    PE 32-row tile positions (weight loads are K=32; the four tiles
    stream concurrently through the array).
  - ACT does one big Exp per group: [128, 2048] PSUM f32 -> SBUF fp16,
    back-to-back across all 62 groups (zero engine gaps in steady state).
  - Reduction over centers on DVE: tensor_reduce has no 2x mode, so
    pairwise fp16 adds (2x capable) halve 256 -> 32 first, then one
    reduce produces y for 1024 points. Output APs stay flat/collapsible
    (DVE is ~10x slower on non-collapsible 4-d output patterns).
  - All bias terms folded on the host into the fp16 hi/lo feature rows;
    no device-side preamble. y accumulates in f32 [128, 496] and drains
    partition-major in overlapped chunks (the last chunk is tiny so the
    post-loop tail is short).

Point -> (lane, slot) mapping n = m*496 + 8g + a keeps the final y DMA
partition-major (1984 B contiguous per partition).
"""

import math

import numpy as np

import concourse.bass as bass
import concourse.bacc as bacc_mod
import concourse.mybir as mybir
from concourse.bass_utils import run_bass_kernel_spmd
from concourse.tile import TileContext

N_CORES = 8
N_TOTAL = 500000
PER_CORE = N_TOTAL // N_CORES  # 62500
M = 256
D = 4
SIGMA = 1.0

NG = 62            # groups per core
BLK = 8            # point-blocks per group (stacked along K)
GPTS = 128 * BLK   # 1024 points per group
NP = NG * GPTS     # 63488 padded points per core
SLOTS = NP // 128  # 496 slots per lane
XCOLS = NG * 128   # 7936 stationary columns

F16 = mybir.dt.float16
F32 = mybir.dt.float32

_CACHE = {}


def _build_nc():
    nc = bacc_mod.Bacc()

    xs_d = nc.dram_tensor("xs", [128, XCOLS], F16, kind="ExternalInput")
    cd_d = nc.dram_tensor("cd", [128, 2 * M], F16, kind="ExternalInput")
    y_d = nc.dram_tensor("y", [NP], F32, kind="ExternalOutput")

    with TileContext(nc) as tc:
        with (
            tc.tile_pool(name="const", bufs=1) as constp,
            tc.tile_pool(name="xsp", bufs=1) as xsp,
            tc.tile_pool(name="expp", bufs=3) as expp,
            tc.tile_pool(name="redp", bufs=2) as redp,
            tc.tile_pool(name="yp", bufs=1) as yp,
            tc.tile_pool(name="psp", bufs=2, space="PSUM") as psp,
        ):
            cd_sb = constp.tile([128, 2 * M], F16)
            nc.sync.dma_start(cd_sb[:], cd_d[:])

            # stationary stream: small first slab so matmul 0 starts early,
            # then 1024-col slabs; all resident (15.5 KB/partition)
            widths = [128, 896] + [1024] * 6 + [768]
            starts = [0]
            for w in widths[:-1]:
                starts.append(starts[-1] + w)
            slabs = []
            for s, (c0, w) in enumerate(zip(starts, widths)):
                t = xsp.tile([128, w], F16, tag=f"xs{s}", bufs=1, name=f"xs{s}")
                nc.sync.dma_start(t[:], xs_d[:, c0 : c0 + w])
                slabs.append(t)

            def slab_of(g):
                c = 128 * g
                for s in range(len(starts) - 1, -1, -1):
                    if c >= starts[s]:
                        return s, c - starts[s]
                raise AssertionError

            ys = yp.tile([128, SLOTS], F32, tag="ys")

            drains = {15: (0, 124), 31: (124, 248), 47: (248, 372),
                      59: (372, 480), 61: (480, 496)}

            for g in range(NG):
                s, col = slab_of(g)
                ps = psp.tile([128, BLK * M], F32, tag="ps")
                # ISA caps matmul moving free at 512: one matmul per pair of
                # point-blocks, PE 32-row tiles so the weight load is K=32.
                for a in range(4):
                    nc.tensor.matmul(
                        ps[:, 512 * a : 512 * (a + 1)],
                        slabs[s][32 * a : 32 * a + 32, col : col + 128],
                        cd_sb[32 * a : 32 * a + 32, :],
                        start=True,
                        stop=True,
                        tile_position=(32 * a, 0),
                    )
                ex = expp.tile([128, BLK * M], F16, tag="ex")
                nc.scalar.activation(
                    ex[:], ps[:], mybir.ActivationFunctionType.Exp
                )
                # tensor_reduce has no DVE 2x mode; halve with tensor_tensor
                # (2x capable) first, reduce only the last 32 lanes.
                t1 = redp.tile([128, BLK * 128], F16, tag="t1")
                e3 = ex[:].rearrange("p (a c) -> p a c", c=M)
                h1 = t1[:].rearrange("p (a c) -> p a c", c=128)
                nc.vector.tensor_tensor(
                    h1, e3[:, :, 0:128], e3[:, :, 128:256], mybir.AluOpType.add
                )
                t2 = redp.tile([128, BLK * 64], F16, tag="t2")
                h2 = t2[:].rearrange("p (a c) -> p a c", c=64)
                nc.vector.tensor_tensor(
                    h2, h1[:, :, 0:64], h1[:, :, 64:128], mybir.AluOpType.add
                )
                t3 = redp.tile([128, BLK * 32], F16, tag="t3")
                h3 = t3[:].rearrange("p (a c) -> p a c", c=32)
                nc.vector.tensor_tensor(
                    h3, h2[:, :, 0:32], h2[:, :, 32:64], mybir.AluOpType.add
                )
                nc.vector.tensor_reduce(
                    ys[:, BLK * g : BLK * (g + 1)],
                    h3,
                    axis=mybir.AxisListType.X,
                    op=mybir.AluOpType.add,
                )
                # drain y directly from ys in overlapped chunks; the final
                # chunk is tiny so the post-loop tail is short
                if g in drains:
                    c0, c1 = drains[g]
                    nc.sync.dma_start(
                        y_d.rearrange("(p f) -> p f", p=128)[:, c0:c1],
                        ys[:, c0:c1],
                    )
    nc.compile()
    return nc


def _host_prep(x, centers, coefficients):
    """Host-side prep: softmax over 256 coefficients, fp16 hi/lo splits,
    per-center and per-point bias folding, streaming layout."""
    x = np.ascontiguousarray(np.asarray(x, dtype=np.float32))
    centers = np.asarray(centers, dtype=np.float32)
    coefficients = np.asarray(coefficients, dtype=np.float32)

    norm_const = np.float32(1.0 / ((2.0 * math.pi) ** (D / 2) * SIGMA**D))
    e = np.exp(coefficients - coefficients.max())
    w = (e / e.sum()).astype(np.float32)
    b = np.log(w * norm_const).astype(np.float32) - 0.5 * (centers**2).sum(axis=1)

    cT = centers.T  # [4, 256]
    c_hi = cT.astype(np.float16)
    c_lo = (cT - c_hi.astype(np.float32)).astype(np.float16)
    b_hi = b.astype(np.float16)
    b_lo = (b - b_hi.astype(np.float32)).astype(np.float16)

    crows = np.empty((16, M), dtype=np.float16)
    crows[0:4] = c_hi
    crows[4:8] = c_lo
    crows[8:12] = c_hi
    crows[12:14] = 1.0
    crows[14] = b_hi
    crows[15] = b_lo

    # [32, 512] two-block diagonal, replicated on all four 32-row bands so
    # band q's slice pairs with stationary rows 32q:32q+32 (blocks 2q, 2q+1)
    cd = np.zeros((128, 2 * M), dtype=np.float16)
    for q in range(4):
        cd[32 * q : 32 * q + 16, 0:M] = crows
        cd[32 * q + 16 : 32 * q + 32, M : 2 * M] = crows

    in_maps = []
    for i in range(N_CORES):
        xs = x[i * PER_CORE : (i + 1) * PER_CORE]
        xp = np.zeros((NP, D), dtype=np.float32)
        xp[:PER_CORE] = xs
        xh = xp.astype(np.float16)
        xl = (xp - xh.astype(np.float32)).astype(np.float16)
        sq = -0.5 * (xp * xp).sum(axis=1)
        sq_hi = sq.astype(np.float16)
        sq_lo = (sq - sq_hi.astype(np.float32)).astype(np.float16)

        feat = np.empty((16, NP), dtype=np.float16)
        feat[0:4] = xh.T      # pairs with c_hi
        feat[4:8] = xh.T      # pairs with c_lo
        feat[8:12] = xl.T     # pairs with c_hi
        feat[12] = sq_hi      # pairs with 1
        feat[13] = sq_lo      # pairs with 1
        feat[14:16] = 1.0     # pairs with b_hi / b_lo

        # n = m*496 + 8g + a  ->  xs_d[16a + k, g*128 + m] = feat[k, n]
        xsd = (
            feat.reshape(16, 128, NG, BLK)
            .transpose(3, 0, 2, 1)
            .reshape(128, XCOLS)
        )
        in_maps.append(
            {"xs": np.ascontiguousarray(xsd), "cd": cd.copy()}
        )
    return in_maps


last_result = None


def kernel(x, centers, coefficients):
    global last_result
    if "nc" not in _CACHE:
        _CACHE["nc"] = _build_nc()
    nc = _CACHE["nc"]
    in_maps = _host_prep(x, centers, coefficients)
    res = run_bass_kernel_spmd(nc, in_maps, core_ids=list(range(N_CORES)))
    last_result = res
    out = []
    for r in res.results:
        y = r["y"][:PER_CORE]
        out.append(y)
    return np.concatenate(out).astype(np.float32)
